# revision 1
# baseline (speedup 1.0000x reference)
"""Trainium2 Bass kernel for nn_DSNet (dense_cnn).

Math: the reference computes
  ref  = conv1d(refer, w_seq, b_seq)            # (1, 512, 32768), k=3 over time
  seq  = concat([ref, x.T], time) -> (65536, 512)
  splits = seq.reshape(32768, 2, 512)
  s    = relu(conv1d(splits, w1, b1))[:, 0, :]  # k=3 over the 512 axis
  s    = relu(s @ w2[:,:,1].T + b2)
  out  = sigmoid(s @ w3[:,:,1].T + b3)          # (32768, 64, 1)

Key folding: for the first 16384 splits (the `ref` half), linear_seq + concat +
split + conv1 collapse into ONE stride-2 4-tap conv applied directly to
`refer` with host-precomputed effective weights Weff[d, i, tau] / beff[d]:
  s1[n, d] = relu( sum_{i, tau} refer[i, 2n + tau - 1] * Weff[d,i,tau] + beff[d] )
For the x half, conv1 along the 512-axis becomes banded matmuls on xT windows.
Everything stays in (channel, split) layout on-chip so no transposes are needed;
matmul2/matmul3 contract over the channel partition dim.

dtypes: conv inputs (refer, x windows, conv weights) are bf16 (halves HBM
traffic; rel err ~1e-3 at the sigmoid output); the psum accumulation and the
s1/h/mm2/mm3 chain stay fp32, with matmuls run as float32r (full-rate PE).

DMA layout: weights are packed into two bundle tensors (bf16 + fp32) and x
windows are stored chunk-major so each n-chunk loads with a single dma_start
(~23 descriptors total; HWDGE costs ~625ns per dma_start, serialized). A few
dummy matmuls + activations at the top warm the PE clock and the ACT tables
while the first DMAs are in flight.

Sharding: splits are sharded 8 ways; core c handles ref-part splits
[2048c, 2048(c+1)) and x-part splits 16384 + [2048c, 2048(c+1)).
"""
import sys

import numpy as np

sys.path.insert(0, "/opt/trn_rl_repo")

D_IN, D_SEQ, D_H, D_OUT = 64, 512, 128, 64
T_REF = 32768
N_CORES = 8
NCHUNK = 512  # splits per n-chunk
WIN = [(128, 126), (128, 126), (128, 126), (128, 126), (10, 8)]  # (K, M) per x window
WB16_COLS = 2 * 4 * 128 + 2 * 126 + 2 * 8 + 512 + 640 + 64  # = 2508
WF32_COLS = 4 + 3  # beff | b2,b3,b1 = 7

_CACHE = {}


def _build_nc():
    import concourse.bacc as bacc
    import concourse.bass as bass
    import concourse.mybir as mybir
    import concourse.tile as tile

    f32 = mybir.dt.float32
    bf16 = mybir.dt.bfloat16
    AF = mybir.ActivationFunctionType
    ALU = mybir.AluOpType
    f32r = mybir.dt.float32r

    nc = bacc.Bacc("TRN2", target_bir_lowering=False, debug=False, num_devices=N_CORES)

    refer_sl = nc.dram_tensor("refer_sl", [D_IN, 4100], bf16, kind="ExternalInput").ap()
    xwc_d = nc.dram_tensor("xwc", [4, 128, 4, 1024], bf16, kind="ExternalInput").ap()
    xw4_d = nc.dram_tensor("xw4", [10, 4096], bf16, kind="ExternalInput").ap()
    wb16_d = nc.dram_tensor("wb16", [128, WB16_COLS], bf16, kind="ExternalInput").ap()
    wf32_d = nc.dram_tensor("wf32", [128, WF32_COLS], f32, kind="ExternalInput").ap()
    out_d = nc.dram_tensor("res", [D_OUT, 4096], f32, kind="ExternalOutput").ap()

    with tile.TileContext(nc) as tc:
        with (
            tc.tile_pool(name="wp", bufs=1) as wp,
            tc.tile_pool(name="dp", bufs=2) as dp,
            tc.tile_pool(name="op", bufs=2) as op,
            tc.tile_pool(name="s1p", bufs=10) as s1p,
            tc.tile_pool(name="hp", bufs=2) as hp,
            tc.tile_pool(name="ppc", bufs=5, space=bass.MemorySpace.PSUM) as ppc,
            tc.tile_pool(name="pph", bufs=2, space=bass.MemorySpace.PSUM) as pph,
            tc.tile_pool(name="ppo", bufs=1, space=bass.MemorySpace.PSUM) as ppo,
        ):
            # PE warmup: dummy matmuls on a memset tile keep the PE busy
            # during the initial DMA wait so real matmuls start at full clock
            warm = wp.tile([1, NCHUNK], bf16)
            nc.gpsimd.memset(warm[:], 0.0)
            wact = wp.tile([1, 16], f32)
            nc.scalar.activation(wact[:], warm[0:1, 0:16], AF.Relu)
            nc.scalar.activation(wact[:], warm[0:1, 0:16], AF.Sigmoid)
            psw = ppo.tile([1, NCHUNK], f32, tag="o", name="psw")
            for _ in range(6):
                nc.tensor.matmul(
                    psw[0:1, :], warm[0:1, 0:1], warm[0:1, :],
                    start=True, stop=True,
                )

            wb16 = wp.tile([128, WB16_COLS], bf16)
            nc.sync.dma_start(wb16[:, 0:1292], wb16_d[:, 0:1292])

            # refer with two tau-shifted copies stacked in the partition dim;
            # loaded in per-chunk column slices interleaved with x chunk loads
            refer2 = wp.tile([128, 4100], bf16)

            def load_ref(b):
                c0, c1 = 1024 * b, 1024 * b + 1026
                nc.sync.dma_start(refer2[0:64, c0:c1], refer_sl[:, c0:c1])
                nc.sync.dma_start(refer2[64:128, c0:c1], refer_sl[:, c0 + 2:c1 + 2])

            load_ref(0)
            wf32 = wp.tile([128, WF32_COLS], f32)
            nc.sync.dma_start(wf32[:], wf32_d[:])
            nc.sync.dma_start(wb16[:, 1292:2508], wb16_d[:, 1292:2508])
            xwin4 = wp.tile([10, 4096], bf16)
            nc.sync.dma_start(xwin4[:], xw4_d[:])

            xt_tiles = {}

            def load_xt(b):
                xt = dp.tile([128, 4, 1024], bf16, tag="xt", name=f"xt_{b}")
                nc.sync.dma_start(xt[:], xwc_d[b])
                xt_tiles[b] = xt

            load_xt(0)
            for b in range(1, 4):
                load_ref(b)
                load_xt(b)

            # weight slice views
            def wefft(t0, q):
                c = t0 * 512 + q * 128
                return wb16[:, c:c + 128]

            def bandA(cp):
                c = 1024 + cp * 126
                return wb16[:, c:c + 126]

            def band4(cp):
                c = 1276 + cp * 8
                return wb16[0:10, c:c + 8]

            def w2r(q):
                return wb16[:, 1292 + q * 128:1292 + (q + 1) * 128]

            def w2x(j):
                return wb16[:, 1804 + j * 128:1804 + (j + 1) * 128]

            w3m = wb16[:, 2444:2508]
            beff = wf32[:, 0:4]
            b2v = wf32[:, 4:5]
            b3v = wf32[0:64, 5:6]
            b1v = wf32[:, 6:7]

            def tail(s1_tiles, sizes, w2_sel, col0, b):
                """mm2 (contract 512) -> relu -> mm3 (128->64) -> sigmoid."""
                hps = pph.tile([128, NCHUNK], f32, tag="h", name=f"hps_{col0}_{b}")
                nct = len(sizes)
                for j in range(nct):
                    m = sizes[j]
                    nc.tensor.matmul(
                        hps[:],
                        w2_sel(j)[0:m, :],
                        s1_tiles[j][0:m, :],
                        start=(j == 0),
                        stop=(j == nct - 1),
                    )
                hsb = hp.tile([128, NCHUNK], bf16, tag="hs", name=f"hsb_{col0}_{b}")
                ops = ppo.tile([64, NCHUNK], f32, tag="o", name=f"ops_{col0}_{b}")
                osb = op.tile([64, NCHUNK], f32, tag="os", name=f"osb_{col0}_{b}")
                c0 = col0 + NCHUNK * b
                nc.vector.tensor_scalar(
                    hsb[:], hps[:], b2v, 0.0, ALU.add, ALU.max
                )
                nc.tensor.matmul(ops[:], w3m, hsb[:], start=True, stop=True)
                nc.scalar.activation(osb[:], ops[:], AF.Sigmoid, bias=b3v)
                nc.sync.dma_start(out_d[:, c0:c0 + NCHUNK], osb[:])

            def ref_chunk(b):
                s1_tiles = []
                for q in range(4):
                    ps = ppc.tile([128, NCHUNK], f32, tag="cv", name=f"psr_{b}_{q}")
                    for t0 in (0, 1):
                        rhs = refer2[:, 1024 * b + t0: 1024 * b + t0 + 2 * NCHUNK: 2]
                        nc.tensor.matmul(
                            ps[:], wefft(t0, q), rhs,
                            start=(t0 == 0), stop=(t0 == 1),
                        )
                    s1 = s1p.tile([128, NCHUNK], bf16, tag="s1", name=f"s1r_{b}_{q}")
                    if q % 2 == 0:
                        nc.scalar.activation(s1[:], ps[:], AF.Relu, bias=beff[:, q:q + 1])
                    else:
                        nc.vector.tensor_scalar(
                            s1[:], ps[:], beff[:, q:q + 1], 0.0, ALU.add, ALU.max
                        )
                    s1_tiles.append(s1)
                tail(s1_tiles, [128, 128, 128, 128], w2r, 0, b)

            def x_chunk(b):
                s1_tiles = []
                xt = xt_tiles[b]
                for j, (K, M) in enumerate(WIN):
                    ps = ppc.tile([128, NCHUNK], f32, tag="cv", name=f"psx_{b}_{j}")
                    for cp in (0, 1):
                        if j < 4:
                            rhs = xt[:, j, cp: 1024: 2]
                            lhsT = bandA(cp)
                        else:
                            rhs = xwin4[:, 1024 * b + cp: 1024 * (b + 1): 2]
                            lhsT = band4(cp)
                        nc.tensor.matmul(
                            ps[0:M, :], lhsT, rhs,
                            start=(cp == 0), stop=(cp == 1),
                        )
                    s1 = s1p.tile([128, NCHUNK], bf16, tag="s1", name=f"s1x_{b}_{j}")
                    if j % 2 == 0:
                        nc.scalar.activation(
                            s1[0:M, :], ps[0:M, :], AF.Relu, bias=b1v[0:M, :]
                        )
                    else:
                        nc.vector.tensor_scalar(
                            s1[0:M, :], ps[0:M, :], b1v[0:M, :], 0.0, ALU.add, ALU.max
                        )
                    s1_tiles.append(s1)
                tail(s1_tiles, [m for _, m in WIN], w2x, 2048, b)

            for b in range(4):
                ref_chunk(b)
                x_chunk(b)

    nc.compile()
    return nc


def _host_prep_weights(w_seq, b_seq, w1, b1, w2, b2, w3, b3):
    import ml_dtypes

    w_seq64 = np.asarray(w_seq, np.float64)
    b_seq64 = np.asarray(b_seq, np.float64)
    w164 = np.asarray(w1, np.float64)

    Weff = np.zeros((D_SEQ, D_IN, 4))
    beff = np.full(D_SEQ, float(np.asarray(b1).reshape(-1)[0]))
    for cc in (0, 1):
        for k in range(3):
            dlo, dhi = max(0, 1 - k), min(D_SEQ, D_SEQ + 1 - k)
            for kk in range(3):
                tau = cc + kk
                Weff[dlo:dhi, :, tau] += (
                    w164[0, cc, k] * w_seq64[dlo + k - 1:dhi + k - 1, :, kk]
                )
    for k in range(3):
        dlo, dhi = max(0, 1 - k), min(D_SEQ, D_SEQ + 1 - k)
        beff[dlo:dhi] += (w164[0, 0, k] + w164[0, 1, k]) * b_seq64[dlo + k - 1:dhi + k - 1]

    # bf16 bundle: wefft (2*4*128) | bandsA (2*126) | band4 (2*8)
    wb16 = np.zeros((128, WB16_COLS), np.float64)
    for t0 in (0, 1):
        for q in range(4):
            c = t0 * 512 + q * 128
            wb16[0:64, c:c + 128] = Weff[128 * q:128 * (q + 1), :, t0].T
            wb16[64:128, c:c + 128] = Weff[128 * q:128 * (q + 1), :, t0 + 2].T
    for cp in (0, 1):
        for m in range(126):
            for k in range(3):
                if m + k < 128:
                    wb16[m + k, 1024 + cp * 126 + m] = w164[0, cp, k]
        for m in range(8):
            for k in range(3):
                if m + k < 10:
                    wb16[m + k, 1276 + cp * 8 + m] = w164[0, cp, k]

    # w2r (4*128) | w2x (5*128) | w3m (64) appended to the bf16 bundle
    w2m = np.asarray(w2, np.float64)[:, :, 1].T  # (512, 128)
    w3m = np.asarray(w3, np.float64)[:, :, 1].T  # (128, 64)
    for q in range(4):
        wb16[:, 1292 + q * 128:1292 + (q + 1) * 128] = w2m[128 * q:128 * (q + 1), :]
    r0 = 0
    for j, (_, M) in enumerate(WIN):
        wb16[0:M, 1804 + j * 128:1804 + j * 128 + 128] = w2m[r0:r0 + M, :]
        r0 += M
    wb16[:, 2444:2508] = w3m
    # fp32 bundle: beff (4) | b2 | b3 | b1
    wf32 = np.zeros((128, WF32_COLS), np.float64)
    for q in range(4):
        wf32[:, q] = beff[128 * q:128 * (q + 1)]
    wf32[:, 4] = np.asarray(b2, np.float64)
    wf32[0:64, 5] = np.asarray(b3, np.float64)
    wf32[:, 6] = float(np.asarray(b1).reshape(-1)[0])

    return (
        np.ascontiguousarray(wb16, ml_dtypes.bfloat16),
        np.ascontiguousarray(wf32, np.float32),
    )


def _host_prep_core(c, refer_bf, x):
    import ml_dtypes

    bf = ml_dtypes.bfloat16
    refer_sl = np.zeros((D_IN, 4100), bf)
    lo, hi = 4096 * c - 1, 4096 * c + 4099
    glo, ghi = max(lo, 0), min(hi, T_REF)
    refer_sl[:, glo - lo:ghi - lo] = refer_bf[0, :, glo:ghi]

    xsl = x[0, 4096 * c:4096 * (c + 1), :]  # (4096, 512) fp32
    xTp = np.zeros((D_SEQ + 2, 4096), bf)
    xTp[1:-1, :] = xsl.T.astype(bf)
    xw = np.zeros((4, 128, 4096), bf)
    for j in range(4):
        xw[j] = xTp[126 * j:126 * j + 128, :]
    # chunk-major: xwc[b, p, j, c] = window_j[p, 1024b + c]
    xwc = np.ascontiguousarray(
        xw.reshape(4, 128, 4, 1024).transpose(2, 1, 0, 3)
    )
    xw4 = np.ascontiguousarray(xTp[504:514, :])
    return refer_sl, xwc, xw4


def kernel(refer, x, w_seq, b_seq, w1, b1, w2, b2, w3, b3):
    import ml_dtypes

    from concourse.bass_utils import run_bass_kernel_spmd

    refer = np.ascontiguousarray(np.asarray(refer), dtype=np.float32)
    x = np.ascontiguousarray(np.asarray(x), dtype=np.float32)
    refer_bf = refer.astype(ml_dtypes.bfloat16)

    if "nc" not in _CACHE:
        _CACHE["nc"] = _build_nc()
    nc = _CACHE["nc"]

    wb16, wf32 = _host_prep_weights(w_seq, b_seq, w1, b1, w2, b2, w3, b3)
    in_maps = []
    for c in range(N_CORES):
        refer_sl, xwc, xw4 = _host_prep_core(c, refer_bf, x)
        in_maps.append(dict(refer_sl=refer_sl, xwc=xwc, xw4=xw4, wb16=wb16, wf32=wf32))

    res = run_bass_kernel_spmd(nc, in_maps, core_ids=list(range(N_CORES)))

    final = np.zeros((32768, D_OUT, 1), np.float32)
    for c in range(N_CORES):
        r = res.results[c]["res"]  # (64, 4096)
        final[2048 * c:2048 * (c + 1), :, 0] = r[:, 0:2048].T
        final[16384 + 2048 * c:16384 + 2048 * (c + 1), :, 0] = r[:, 2048:4096].T
    return final



# revision 6
# speedup vs baseline: 1.2583x; 1.2583x over previous
"""Trainium2 Bass kernel for nn_DSNet (dense_cnn) — fp8 DoubleRow version.

Math: the reference computes
  ref  = conv1d(refer, w_seq, b_seq)            # (1, 512, 32768), k=3 over time
  seq  = concat([ref, x.T], time) -> (65536, 512)
  splits = seq.reshape(32768, 2, 512)
  s1   = relu(conv1d(splits, w1, b1))[:, 0, :]  # k=3 over the 512 axis
  h    = relu(s1 @ w2[:,:,1].T + b2)
  out  = sigmoid(h @ w3[:,:,1].T + b3)          # (32768, 64, 1)

Folding: for ref-half splits, linear_seq + concat + split + conv1 collapse into a
stride-2 4-tap conv on `refer` with host-precomputed Weff[d,i,tau] / beff[d]
(weight-only math). For the x half, conv1 along the 512 axis becomes banded
matmuls on 128-aligned xT windows plus tiny edge-fix matmuls.

Speed: all conv + mm2 matmuls run in fp8 e4m3 with MatmulPerfMode.DoubleRow
(2 contraction rows per PE cell -> 0.5 cycles/col). lhsT layout [K, 2, M],
rhs [K, 2, N]; psum stays fp32. Biases are folded into the conv matmuls via a
constant-1 rhs row so one relu can serve two d-blocks with different biases.
s1 is stored as fp8 [128, 2, 512] (two d-blocks per partition) so mm2 also
runs DoubleRow; mm3 runs bf16. Conv psums are merged pairwise into [128, 1024]
(2 banks) so each relu instruction covers two d-blocks; the two mm3 outputs
share one psum bank (ref rows 0:64, x rows 64:128) so one sigmoid covers both.

Elementwise (the bottleneck at ~12us/core) is split between ACT and DVE;
GPSIMD cannot read PSUM so it cannot help.

Sharding: splits sharded 8 ways; core c handles ref splits [2048c, 2048(c+1))
and x splits 16384 + [2048c, 2048(c+1)). Output per core [128, 2048] f32:
rows 0:64 ref outs, rows 64:128 x outs.
"""
import sys

import numpy as np

sys.path.insert(0, "/opt/trn_rl_repo")

D_IN, D_SEQ, D_H, D_OUT = 64, 512, 128, 64
T_REF = 32768
N_CORES = 8
NCH = 512  # splits per chunk
S1_FP8 = True  # fall back to bf16 s1 + bf16 mm2 if fp8 rounding hurts accuracy

_CACHE = {}


def _build_nc():
    import concourse.bacc as bacc
    import concourse.bass as bass
    import concourse.mybir as mybir
    import concourse.tile as tile

    f32 = mybir.dt.float32
    bf16 = mybir.dt.bfloat16
    e4 = mybir.dt.float8e4
    AF = mybir.ActivationFunctionType
    ALU = mybir.AluOpType
    DR = mybir.MatmulPerfMode.DoubleRow
    s1dt = e4 if S1_FP8 else bf16

    nc = bacc.Bacc("TRN2", target_bir_lowering=False, debug=False, num_devices=N_CORES)

    refer_d = nc.dram_tensor("refer_sl", [65, 4100], e4, kind="ExternalInput").ap()
    xwc_d = nc.dram_tensor("xwc", [4, 128, 4, 1024], e4, kind="ExternalInput").ap()
    edge_d = nc.dram_tensor("edge", [3, 4, 4, 1024], e4, kind="ExternalInput").ap()
    # ref conv lhsT: [65 part][i 2][(q,g) 8][m 128]
    refw_d = nc.dram_tensor("refw", [65, 2, 8, 128], e4, kind="ExternalInput").ap()
    # x conv lhsT main + edge, mm2 lhsT (2 tiles), packed in one fp8 bundle:
    # [128 part][i 2][slot 4][m 128]: slot0=xmain, slot1=w2 t0, slot2=w2 t1, slot3=edge(rows 0:3)
    wx_d = nc.dram_tensor("wx", [128, 2, 4, 128], e4, kind="ExternalInput").ap()
    w3_d = nc.dram_tensor("w3", [128, 64], bf16, kind="ExternalInput").ap()
    wf_d = nc.dram_tensor("wf", [128, 2], f32, kind="ExternalInput").ap()
    out_d = nc.dram_tensor("res", [128, 2048], f32, kind="ExternalOutput").ap()

    with tile.TileContext(nc) as tc:
        with (
            tc.tile_pool(name="wp", bufs=1) as wp,
            tc.tile_pool(name="dp", bufs=2) as dp,
            tc.tile_pool(name="s1p", bufs=6) as s1p,
            tc.tile_pool(name="hp", bufs=2) as hp,
            tc.tile_pool(name="op", bufs=2) as op,
            tc.tile_pool(name="ppc", bufs=2, space=bass.MemorySpace.PSUM) as ppc,
            tc.tile_pool(name="pph", bufs=1, space=bass.MemorySpace.PSUM) as pph,
            tc.tile_pool(name="ppo", bufs=2, space=bass.MemorySpace.PSUM) as ppo,
        ):
            # --- PE clock + ACT table warmup while first DMAs land
            warm = wp.tile([1, NCH], bf16)
            nc.gpsimd.memset(warm[:], 0.0)
            wact = wp.tile([1, 16], f32)
            nc.scalar.activation(wact[:], warm[0:1, 0:16], AF.Relu)
            nc.scalar.activation(wact[:], warm[0:1, 0:16], AF.Sigmoid)
            psw = ppo.tile([128, NCH], f32, tag="o", name="psw")
            for _ in range(6):
                nc.tensor.matmul(
                    psw[0:1, :], warm[0:1, 0:1], warm[0:1, :], start=True, stop=True
                )

            # --- weight + data loads
            refw = wp.tile([65, 2, 8, 128], e4)
            nc.sync.dma_start(refw[:], refw_d)
            refer_sl = wp.tile([65, 4100], e4)
            nc.sync.dma_start(refer_sl[:, 0:2052], refer_d[:, 0:2052])
            wx = wp.tile([128, 2, 4, 128], e4)
            nc.sync.dma_start(wx[:], wx_d)

            xt_tiles = {}

            def load_xt(b):
                xt = dp.tile([128, 4, 1024], e4, tag="xt", name=f"xt_{b}")
                nc.sync.dma_start(xt[:], xwc_d[b])
                xt_tiles[b] = xt

            load_xt(0)
            w3 = wp.tile([128, 64], bf16)
            nc.sync.dma_start(w3[:], w3_d)
            wf = wp.tile([128, 2], f32)
            nc.sync.dma_start(wf[:], wf_d)
            edge = wp.tile([3, 4, 4, 1024], e4)
            nc.sync.dma_start(edge[:], edge_d)
            load_xt(1)
            nc.sync.dma_start(refer_sl[:, 2052:4100], refer_d[:, 2052:4100])
            load_xt(2)
            load_xt(3)

            b2v = wf[:, 0:1]
            b3v = wf[:, 1:2]

            def ref_conv(b, t):
                """Merged conv psum for ref d-blocks (2t, 2t+1) of chunk b."""
                ps = ppc.tile([128, 1024], f32, tag="cv", name=f"psr_{b}_{t}")
                for qh in (0, 1):
                    q = 2 * t + qh
                    reg = ps[:, 512 * qh:512 * qh + 512]
                    for g in (0, 1):
                        base = 1024 * b + 2 * g
                        np_ = 65 if g == 0 else 64
                        rhs = refer_sl[0:np_, base:base + 1024].rearrange(
                            "p (n i) -> p i n", i=2
                        )
                        nc.tensor.matmul(
                            reg, refw[0:np_, :, 2 * q + g, :], rhs,
                            start=(g == 0), stop=(g == 1), perf_mode=DR,
                        )
                return ps

            def x_conv(b, t):
                """Merged conv psum for x windows (2t, 2t+1) of chunk b."""
                xt = xt_tiles[b]
                ps = ppc.tile([128, 1024], f32, tag="cv", name=f"psx_{b}_{t}")
                for jh in (0, 1):
                    j = 2 * t + jh
                    reg = ps[:, 512 * jh:512 * jh + 512]
                    rhs = xt[:, j, :].rearrange("p (n i) -> p i n", i=2)
                    nc.tensor.matmul(
                        reg, wx[:, :, 0, :], rhs, start=True, stop=False, perf_mode=DR
                    )
                    erhs = edge[:, j, b, :].rearrange("p (n i) -> p i n", i=2)
                    nc.tensor.matmul(
                        reg, wx[0:3, :, 3, :], erhs, start=False, stop=True,
                        perf_mode=DR,
                    )
                return ps

            def s1_relu(ps, eng, name):
                s1 = s1p.tile([128, 2, 512], s1dt, tag="s1", name=name)
                flat = s1.rearrange("p a b -> p (a b)")
                if eng == 0:
                    nc.scalar.activation(flat, ps[:], AF.Relu)
                else:
                    nc.vector.tensor_scalar(flat, ps[:], 0.0, None, ALU.max)
                return s1

            def mm2(ph, half, s1_tiles):
                reg = ph[:, 512 * half:512 * half + 512]
                for t in (0, 1):
                    if S1_FP8:
                        nc.tensor.matmul(
                            reg, wx[:, :, 1 + t, :], s1_tiles[t][:],
                            start=(t == 0), stop=(t == 1), perf_mode=DR,
                        )
                    else:
                        for i in (0, 1):
                            nc.tensor.matmul(
                                reg, wx[:, i, 1 + t, :], s1_tiles[t][:, i, :],
                                start=(t == 0 and i == 0), stop=(t == 1 and i == 1),
                            )

            def pair(b):
                eng = b % 2
                # ref conv + s1
                s1r = []
                for t in (0, 1):
                    ps = ref_conv(b, t)
                    s1r.append(s1_relu(ps, (eng + t) % 2, f"s1r_{b}_{t}"))
                ph = pph.tile([128, 1024], f32, tag="h", name=f"ph_{b}")
                mm2(ph, 0, s1r)
                # x conv + s1
                s1x = []
                for t in (0, 1):
                    ps = x_conv(b, t)
                    s1x.append(s1_relu(ps, (eng + t + 1) % 2, f"s1x_{b}_{t}"))
                mm2(ph, 1, s1x)
                # merged h relu (+b2) -> bf16
                hsb = hp.tile([128, 1024], bf16, tag="hs", name=f"hsb_{b}")
                if eng == 0:
                    nc.scalar.activation(hsb[:], ph[:], AF.Relu, bias=b2v)
                else:
                    nc.vector.tensor_scalar(hsb[:], ph[:], b2v, 0.0, ALU.add, ALU.max)
                # mm3 ref -> rows 0:64, x -> rows 64:128 of one psum bank
                po = ppo.tile([128, 512], f32, tag="o", name=f"po_{b}")
                nc.tensor.matmul(po[0:64, :], w3[:, :], hsb[:, 0:512],
                                 start=True, stop=True)
                nc.tensor.matmul(po[64:128, :], w3[:, :], hsb[:, 512:1024],
                                 start=True, stop=True)
                osb = op.tile([128, 512], f32, tag="os", name=f"osb_{b}")
                nc.scalar.activation(osb[:], po[:], AF.Sigmoid, bias=b3v)
                nc.sync.dma_start(out_d[:, 512 * b:512 * b + 512], osb[:])

            for b in range(4):
                pair(b)

    nc.compile()
    return nc


def _host_prep_weights(w_seq, b_seq, w1, b1, w2, b2, w3, b3):
    import ml_dtypes

    e4 = ml_dtypes.float8_e4m3
    bf = ml_dtypes.bfloat16

    w_seq64 = np.asarray(w_seq, np.float64)
    b_seq64 = np.asarray(b_seq, np.float64)
    w164 = np.asarray(w1, np.float64)
    b1f = float(np.asarray(b1).reshape(-1)[0])

    # Effective stride-2 4-tap conv weights for the ref half
    Weff = np.zeros((D_SEQ, D_IN, 4))
    beff = np.full(D_SEQ, b1f)
    for cc in (0, 1):
        for k in range(3):
            dlo, dhi = max(0, 1 - k), min(D_SEQ, D_SEQ + 1 - k)
            for kk in range(3):
                tau = cc + kk
                Weff[dlo:dhi, :, tau] += (
                    w164[0, cc, k] * w_seq64[dlo + k - 1:dhi + k - 1, :, kk]
                )
    for k in range(3):
        dlo, dhi = max(0, 1 - k), min(D_SEQ, D_SEQ + 1 - k)
        beff[dlo:dhi] += (w164[0, 0, k] + w164[0, 1, k]) * b_seq64[dlo + k - 1:dhi + k - 1]

    # ref conv lhsT [65, 2, 8, 128]: (q, g) slot 2q+g; group i -> tau 2g+i
    refw = np.zeros((65, 2, 8, 128), np.float64)
    for q in range(4):
        for g in (0, 1):
            for i in (0, 1):
                refw[0:64, i, 2 * q + g, :] = Weff[128 * q:128 * (q + 1), :, 2 * g + i].T
        refw[64, 0, 2 * q + 0, :] = beff[128 * q:128 * (q + 1)]

    # x conv lhsT main [128, 2, 128]: lhsT[m+k, c, m] = w1[c, k] for m+k<=127
    xmain = np.zeros((128, 2, 128), np.float64)
    for c in (0, 1):
        for k in range(3):
            for m in range(128):
                if m + k <= 127:
                    xmain[m + k, c, m] = w164[0, c, k]
    # x conv lhsT edge [3, 2, 128]: row0 = first edge input (d'=128j+127):
    #   m=126 tap k=2, m=127 tap k=1; row1 = second (d'=128j+128): m=127 k=2;
    #   row2 = ones row: b1 in group 0
    xedge = np.zeros((3, 2, 128), np.float64)
    for c in (0, 1):
        xedge[0, c, 126] = w164[0, c, 2]
        xedge[0, c, 127] = w164[0, c, 1]
        xedge[1, c, 127] = w164[0, c, 2]
    xedge[2, 0, :] = b1f

    # mm2 lhsT tiles [128, 2, 128] x2: w2t[p, i, t, m] = w2m[128(2t+i)+p, m]
    w2m = np.asarray(w2, np.float64)[:, :, 1].T  # (512, 128)
    wx = np.zeros((128, 2, 4, 128), np.float64)
    wx[:, :, 0, :] = xmain
    for t in (0, 1):
        for i in (0, 1):
            wx[:, i, 1 + t, :] = w2m[128 * (2 * t + i):128 * (2 * t + i + 1), :]
    wx[0:3, :, 3, :] = xedge

    w3m = np.asarray(w3, np.float64)[:, :, 1].T  # (128, 64)

    wf = np.zeros((128, 2), np.float64)
    wf[:, 0] = np.asarray(b2, np.float64)
    wf[0:64, 1] = np.asarray(b3, np.float64)
    wf[64:128, 1] = np.asarray(b3, np.float64)

    return (
        np.ascontiguousarray(refw, e4),
        np.ascontiguousarray(wx, e4),
        np.ascontiguousarray(w3m, bf),
        np.ascontiguousarray(wf, np.float32),
    )


def _host_prep_core(c, refer8, x):
    import ml_dtypes

    e4 = ml_dtypes.float8_e4m3
    # refer_sl [65, 4100]: rows 0:64 refer cols [4096c-1, 4096c+4099), row 64 ones
    refer_sl = np.zeros((65, 4100), e4)
    lo, hi = 4096 * c - 1, 4096 * c + 4099
    glo, ghi = max(lo, 0), min(hi, T_REF)
    refer_sl[0:64, glo - lo:ghi - lo] = refer8[0, :, glo:ghi]
    refer_sl[64, :] = np.float64(1.0)

    # x windows: xTl[d, t] = x[4096c + t, d]
    xsl = x[0, 4096 * c:4096 * (c + 1), :]  # (4096, 512) f32
    xT8 = xsl.T.astype(e4)                  # (512, 4096)
    xwc = np.zeros((4, 128, 4, 1024), e4)
    for j in range(4):
        d0 = 128 * j - 1
        rlo = max(d0, 0)
        rhi = min(d0 + 128, D_SEQ)
        xwc[:, rlo - d0:rhi - d0, j, :] = (
            xT8[rlo:rhi, :].reshape(rhi - rlo, 4, 1024).transpose(1, 0, 2)
        )
    edge = np.zeros((3, 4, 4, 1024), e4)
    for j in range(4):
        edge[0, j, :, :] = xT8[128 * j + 127, :].reshape(4, 1024)
        if 128 * j + 128 < D_SEQ:
            edge[1, j, :, :] = xT8[128 * j + 128, :].reshape(4, 1024)
        edge[2, j, :, :] = np.float64(1.0)
    return refer_sl, xwc, edge


def kernel(refer, x, w_seq, b_seq, w1, b1, w2, b2, w3, b3):
    import ml_dtypes

    from concourse.bass_utils import run_bass_kernel_spmd

    refer = np.ascontiguousarray(np.asarray(refer), dtype=np.float32)
    x = np.ascontiguousarray(np.asarray(x), dtype=np.float32)
    refer8 = refer.astype(ml_dtypes.float8_e4m3)

    if "nc" not in _CACHE:
        _CACHE["nc"] = _build_nc()
    nc = _CACHE["nc"]

    refw, wx, w3m, wf = _host_prep_weights(w_seq, b_seq, w1, b1, w2, b2, w3, b3)
    in_maps = []
    for c in range(N_CORES):
        refer_sl, xwc, edge = _host_prep_core(c, refer8, x)
        in_maps.append(dict(
            refer_sl=refer_sl, xwc=xwc, edge=edge, refw=refw, wx=wx, w3=w3m, wf=wf
        ))

    res = run_bass_kernel_spmd(nc, in_maps, core_ids=list(range(N_CORES)))

    final = np.zeros((32768, D_OUT, 1), np.float32)
    for c in range(N_CORES):
        r = res.results[c]["res"]  # (128, 2048)
        final[2048 * c:2048 * (c + 1), :, 0] = r[0:64, :].T
        final[16384 + 2048 * c:16384 + 2048 * (c + 1), :, 0] = r[64:128, :].T
    return final


# revision 8
# speedup vs baseline: 1.3475x; 1.0709x over previous
"""Trainium2 Bass kernel for nn_DSNet (dense_cnn) — fp8 DoubleRow version.

Math: the reference computes
  ref  = conv1d(refer, w_seq, b_seq)            # (1, 512, 32768), k=3 over time
  seq  = concat([ref, x.T], time) -> (65536, 512)
  splits = seq.reshape(32768, 2, 512)
  s1   = relu(conv1d(splits, w1, b1))[:, 0, :]  # k=3 over the 512 axis
  h    = relu(s1 @ w2[:,:,1].T + b2)
  out  = sigmoid(h @ w3[:,:,1].T + b3)          # (32768, 64, 1)

Folding: for ref-half splits, linear_seq + concat + split + conv1 collapse into a
stride-2 4-tap conv on `refer` with host-precomputed Weff[d,i,tau] / beff[d]
(weight-only math). For the x half, conv1 along the 512 axis becomes banded
matmuls on 128-aligned xT windows plus tiny edge-fix matmuls.

Speed: all conv + mm2 matmuls run in fp8 e4m3 with MatmulPerfMode.DoubleRow
(2 contraction rows per PE cell -> 0.5 cycles/col). lhsT layout [K, 2, M],
rhs [K, 2, N]; psum stays fp32. Biases are folded into the conv matmuls via a
constant-1 rhs row so one relu can serve two d-blocks with different biases.
s1 is stored as fp8 [128, 2, 512] (two d-blocks per partition) so mm2 also
runs DoubleRow; mm3 runs bf16. Conv psums are merged pairwise into [128, 1024]
(2 banks) so each relu instruction covers two d-blocks; the two mm3 outputs
share one psum bank (ref rows 0:64, x rows 64:128) so one sigmoid covers both.

Elementwise (the bottleneck at ~12us/core) is split between ACT and DVE;
GPSIMD cannot read PSUM so it cannot help.

Sharding: splits sharded 8 ways; core c handles ref splits [2048c, 2048(c+1))
and x splits 16384 + [2048c, 2048(c+1)). Output per core [128, 2048] f32:
rows 0:64 ref outs, rows 64:128 x outs.
"""
import sys

import numpy as np

sys.path.insert(0, "/opt/trn_rl_repo")

D_IN, D_SEQ, D_H, D_OUT = 64, 512, 128, 64
T_REF = 32768
N_CORES = 8
NCH = 512  # splits per chunk
S1_FP8 = True  # fall back to bf16 s1 + bf16 mm2 if fp8 rounding hurts accuracy

_CACHE = {}


def _build_nc():
    import concourse.bacc as bacc
    import concourse.bass as bass
    import concourse.mybir as mybir
    import concourse.tile as tile

    f32 = mybir.dt.float32
    bf16 = mybir.dt.bfloat16
    e4 = mybir.dt.float8e4
    AF = mybir.ActivationFunctionType
    ALU = mybir.AluOpType
    DR = mybir.MatmulPerfMode.DoubleRow
    s1dt = e4 if S1_FP8 else bf16

    nc = bacc.Bacc("TRN2", target_bir_lowering=False, debug=False, num_devices=N_CORES)

    refer_d = nc.dram_tensor("refer_sl", [65, 4100], e4, kind="ExternalInput").ap()
    xwc_d = nc.dram_tensor("xwc", [4, 128, 4, 1024], e4, kind="ExternalInput").ap()
    edge_d = nc.dram_tensor("edge", [3, 4, 4, 1024], e4, kind="ExternalInput").ap()
    # ref conv lhsT: [65 part][i 2][(q,g) 8][m 128]
    refw_d = nc.dram_tensor("refw", [65, 2, 8, 128], e4, kind="ExternalInput").ap()
    # x conv lhsT main + edge, mm2 lhsT (2 tiles), packed in one fp8 bundle:
    # [128 part][i 2][slot 4][m 128]: slot0=xmain, slot1=w2 t0, slot2=w2 t1, slot3=edge(rows 0:3)
    wx_d = nc.dram_tensor("wx", [128, 2, 4, 128], e4, kind="ExternalInput").ap()
    w3_d = nc.dram_tensor("w3", [128, 64], bf16, kind="ExternalInput").ap()
    wf_d = nc.dram_tensor("wf", [128, 2], f32, kind="ExternalInput").ap()
    out_d = nc.dram_tensor("res", [128, 2048], f32, kind="ExternalOutput").ap()

    with tile.TileContext(nc) as tc:
        with (
            tc.tile_pool(name="wp", bufs=1) as wp,
            tc.tile_pool(name="dp", bufs=2) as dp,
            tc.tile_pool(name="s1p", bufs=6) as s1p,
            tc.tile_pool(name="hp", bufs=2) as hp,
            tc.tile_pool(name="op", bufs=2) as op,
            tc.tile_pool(name="ppc", bufs=3, space=bass.MemorySpace.PSUM) as ppc,
            tc.tile_pool(name="pph", bufs=1, space=bass.MemorySpace.PSUM) as pph,
        ):
            # --- PE clock + ACT table warmup while first DMAs land
            warm = wp.tile([1, NCH], bf16)
            nc.gpsimd.memset(warm[:], 0.0)
            wact = wp.tile([1, 16], f32)
            nc.scalar.activation(wact[:], warm[0:1, 0:16], AF.Relu)
            nc.scalar.activation(wact[:], warm[0:1, 0:16], AF.Sigmoid)
            psw = ppc.tile([128, 1024], f32, tag="cv", name="psw")
            for _ in range(6):
                nc.tensor.matmul(
                    psw[0:1, 0:512], warm[0:1, 0:1], warm[0:1, :], start=True,
                    stop=True,
                )

            # --- weight + data loads
            refw = wp.tile([65, 2, 8, 128], e4)
            nc.sync.dma_start(refw[:], refw_d)
            refer_sl = wp.tile([65, 4100], e4)
            nc.sync.dma_start(refer_sl[:, 0:2052], refer_d[:, 0:2052])
            wx = wp.tile([128, 2, 4, 128], e4)
            nc.sync.dma_start(wx[:], wx_d)

            xt_tiles = {}

            def load_xt(b):
                xt = dp.tile([128, 4, 1024], e4, tag="xt", name=f"xt_{b}")
                nc.sync.dma_start(xt[:], xwc_d[b])
                xt_tiles[b] = xt

            load_xt(0)
            w3 = wp.tile([128, 64], bf16)
            nc.sync.dma_start(w3[:], w3_d)
            wf = wp.tile([128, 2], f32)
            nc.sync.dma_start(wf[:], wf_d)
            edge = wp.tile([3, 4, 4, 1024], e4)
            nc.sync.dma_start(edge[:], edge_d)
            load_xt(1)
            nc.sync.dma_start(refer_sl[:, 2052:4100], refer_d[:, 2052:4100])
            load_xt(2)
            load_xt(3)

            b2v = wf[:, 0:1]
            b3v = wf[:, 1:2]

            def ref_conv(b, t):
                """Merged conv psum for ref d-blocks (2t, 2t+1) of chunk b."""
                ps = ppc.tile([128, 1024], f32, tag="cv", name=f"psr_{b}_{t}")
                for qh in (0, 1):
                    q = 2 * t + qh
                    reg = ps[:, 512 * qh:512 * qh + 512]
                    for g in (0, 1):
                        base = 1024 * b + 2 * g
                        np_ = 65 if g == 0 else 64
                        rhs = refer_sl[0:np_, base:base + 1024].rearrange(
                            "p (n i) -> p i n", i=2
                        )
                        nc.tensor.matmul(
                            reg, refw[0:np_, :, 2 * q + g, :], rhs,
                            start=(g == 0), stop=(g == 1), perf_mode=DR,
                        )
                return ps

            def x_conv(b, t):
                """Merged conv psum for x windows (2t, 2t+1) of chunk b."""
                xt = xt_tiles[b]
                ps = ppc.tile([128, 1024], f32, tag="cv", name=f"psx_{b}_{t}")
                for jh in (0, 1):
                    j = 2 * t + jh
                    reg = ps[:, 512 * jh:512 * jh + 512]
                    rhs = xt[:, j, :].rearrange("p (n i) -> p i n", i=2)
                    nc.tensor.matmul(
                        reg, wx[:, :, 0, :], rhs, start=True, stop=False, perf_mode=DR
                    )
                    erhs = edge[:, j, b, :].rearrange("p (n i) -> p i n", i=2)
                    nc.tensor.matmul(
                        reg, wx[0:3, :, 3, :], erhs, start=False, stop=True,
                        perf_mode=DR,
                    )
                return ps

            def s1_relu(ps, eng, name):
                s1 = s1p.tile([128, 2, 512], s1dt, tag="s1", name=name)
                flat = s1.rearrange("p a b -> p (a b)")
                if eng == 0:
                    nc.scalar.activation(flat, ps[:], AF.Relu)
                else:
                    nc.vector.tensor_scalar(flat, ps[:], 0.0, None, ALU.max)
                return s1

            def mm2(ph, half, s1_tiles):
                reg = ph[:, 512 * half:512 * half + 512]
                for t in (0, 1):
                    if S1_FP8:
                        nc.tensor.matmul(
                            reg, wx[:, :, 1 + t, :], s1_tiles[t][:],
                            start=(t == 0), stop=(t == 1), perf_mode=DR,
                        )
                    else:
                        for i in (0, 1):
                            nc.tensor.matmul(
                                reg, wx[:, i, 1 + t, :], s1_tiles[t][:, i, :],
                                start=(t == 0 and i == 0), stop=(t == 1 and i == 1),
                            )

            # Software-pipelined emission: engine queues are in-order, so the
            # tail of pair b-1 (mm3/sigmoid/dma, whose deps finish late) is
            # emitted inside iteration b between ready relu work, and the
            # mm3 output reuses the h psum banks (WAR ordered by Tile).
            ph_tiles = {}
            hs_tiles = {}

            def tail(p):
                ph, hsb = ph_tiles.pop(p), hs_tiles.pop(p)
                po = ph[:, 0:512]
                nc.tensor.matmul(po[0:64, :], w3[:, :], hsb[:, 0:512],
                                 start=True, stop=True)
                nc.tensor.matmul(po[64:128, :], w3[:, :], hsb[:, 512:1024],
                                 start=True, stop=True)
                osb = op.tile([128, 512], f32, tag="os", name=f"osb_{p}")
                nc.scalar.activation(osb[:], po, AF.Sigmoid, bias=b3v)
                nc.sync.dma_start(out_d[:, 512 * p:512 * p + 512], osb[:])

            for b in range(4):
                # ref convs + their relus (ACT gets t0, DVE t1)
                psr = [ref_conv(b, t) for t in (0, 1)]
                s1r = [
                    s1_relu(psr[0], 0, f"s1r_{b}_0"),
                    s1_relu(psr[1], 1, f"s1r_{b}_1"),
                ]
                # tail of previous pair: deps (h relu of b-1) already draining
                if b >= 1:
                    tail(b - 1)
                # x convs + relus (DVE t0, ACT t1)
                psx = [x_conv(b, t) for t in (0, 1)]
                s1x = [
                    s1_relu(psx[0], 1, f"s1x_{b}_0"),
                    s1_relu(psx[1], 0, f"s1x_{b}_1"),
                ]
                ph = pph.tile([128, 1024], f32, tag="h", name=f"ph_{b}")
                ph_tiles[b] = ph
                mm2(ph, 0, s1r)
                mm2(ph, 1, s1x)
                # merged h relu (+b2) -> bf16; alternate engine per pair
                hsb = hp.tile([128, 1024], bf16, tag="hs", name=f"hsb_{b}")
                hs_tiles[b] = hsb
                if b % 2 == 0:
                    nc.vector.tensor_scalar(hsb[:], ph[:], b2v, 0.0, ALU.add, ALU.max)
                else:
                    nc.scalar.activation(hsb[:], ph[:], AF.Relu, bias=b2v)
            tail(3)

    nc.compile()
    return nc


def _host_prep_weights(w_seq, b_seq, w1, b1, w2, b2, w3, b3):
    import ml_dtypes

    e4 = ml_dtypes.float8_e4m3
    bf = ml_dtypes.bfloat16

    w_seq64 = np.asarray(w_seq, np.float64)
    b_seq64 = np.asarray(b_seq, np.float64)
    w164 = np.asarray(w1, np.float64)
    b1f = float(np.asarray(b1).reshape(-1)[0])

    # Effective stride-2 4-tap conv weights for the ref half
    Weff = np.zeros((D_SEQ, D_IN, 4))
    beff = np.full(D_SEQ, b1f)
    for cc in (0, 1):
        for k in range(3):
            dlo, dhi = max(0, 1 - k), min(D_SEQ, D_SEQ + 1 - k)
            for kk in range(3):
                tau = cc + kk
                Weff[dlo:dhi, :, tau] += (
                    w164[0, cc, k] * w_seq64[dlo + k - 1:dhi + k - 1, :, kk]
                )
    for k in range(3):
        dlo, dhi = max(0, 1 - k), min(D_SEQ, D_SEQ + 1 - k)
        beff[dlo:dhi] += (w164[0, 0, k] + w164[0, 1, k]) * b_seq64[dlo + k - 1:dhi + k - 1]

    # ref conv lhsT [65, 2, 8, 128]: (q, g) slot 2q+g; group i -> tau 2g+i
    refw = np.zeros((65, 2, 8, 128), np.float64)
    for q in range(4):
        for g in (0, 1):
            for i in (0, 1):
                refw[0:64, i, 2 * q + g, :] = Weff[128 * q:128 * (q + 1), :, 2 * g + i].T
        refw[64, 0, 2 * q + 0, :] = beff[128 * q:128 * (q + 1)]

    # x conv lhsT main [128, 2, 128]: lhsT[m+k, c, m] = w1[c, k] for m+k<=127
    xmain = np.zeros((128, 2, 128), np.float64)
    for c in (0, 1):
        for k in range(3):
            for m in range(128):
                if m + k <= 127:
                    xmain[m + k, c, m] = w164[0, c, k]
    # x conv lhsT edge [3, 2, 128]: row0 = first edge input (d'=128j+127):
    #   m=126 tap k=2, m=127 tap k=1; row1 = second (d'=128j+128): m=127 k=2;
    #   row2 = ones row: b1 in group 0
    xedge = np.zeros((3, 2, 128), np.float64)
    for c in (0, 1):
        xedge[0, c, 126] = w164[0, c, 2]
        xedge[0, c, 127] = w164[0, c, 1]
        xedge[1, c, 127] = w164[0, c, 2]
    xedge[2, 0, :] = b1f

    # mm2 lhsT tiles [128, 2, 128] x2: w2t[p, i, t, m] = w2m[128(2t+i)+p, m]
    w2m = np.asarray(w2, np.float64)[:, :, 1].T  # (512, 128)
    wx = np.zeros((128, 2, 4, 128), np.float64)
    wx[:, :, 0, :] = xmain
    for t in (0, 1):
        for i in (0, 1):
            wx[:, i, 1 + t, :] = w2m[128 * (2 * t + i):128 * (2 * t + i + 1), :]
    wx[0:3, :, 3, :] = xedge

    w3m = np.asarray(w3, np.float64)[:, :, 1].T  # (128, 64)

    wf = np.zeros((128, 2), np.float64)
    wf[:, 0] = np.asarray(b2, np.float64)
    wf[0:64, 1] = np.asarray(b3, np.float64)
    wf[64:128, 1] = np.asarray(b3, np.float64)

    return (
        np.ascontiguousarray(refw, e4),
        np.ascontiguousarray(wx, e4),
        np.ascontiguousarray(w3m, bf),
        np.ascontiguousarray(wf, np.float32),
    )


def _host_prep_core(c, refer8, x):
    import ml_dtypes

    e4 = ml_dtypes.float8_e4m3
    # refer_sl [65, 4100]: rows 0:64 refer cols [4096c-1, 4096c+4099), row 64 ones
    refer_sl = np.zeros((65, 4100), e4)
    lo, hi = 4096 * c - 1, 4096 * c + 4099
    glo, ghi = max(lo, 0), min(hi, T_REF)
    refer_sl[0:64, glo - lo:ghi - lo] = refer8[0, :, glo:ghi]
    refer_sl[64, :] = np.float64(1.0)

    # x windows: xTl[d, t] = x[4096c + t, d]
    xsl = x[0, 4096 * c:4096 * (c + 1), :]  # (4096, 512) f32
    xT8 = xsl.T.astype(e4)                  # (512, 4096)
    xwc = np.zeros((4, 128, 4, 1024), e4)
    for j in range(4):
        d0 = 128 * j - 1
        rlo = max(d0, 0)
        rhi = min(d0 + 128, D_SEQ)
        xwc[:, rlo - d0:rhi - d0, j, :] = (
            xT8[rlo:rhi, :].reshape(rhi - rlo, 4, 1024).transpose(1, 0, 2)
        )
    edge = np.zeros((3, 4, 4, 1024), e4)
    for j in range(4):
        edge[0, j, :, :] = xT8[128 * j + 127, :].reshape(4, 1024)
        if 128 * j + 128 < D_SEQ:
            edge[1, j, :, :] = xT8[128 * j + 128, :].reshape(4, 1024)
        edge[2, j, :, :] = np.float64(1.0)
    return refer_sl, xwc, edge


def kernel(refer, x, w_seq, b_seq, w1, b1, w2, b2, w3, b3):
    import ml_dtypes

    from concourse.bass_utils import run_bass_kernel_spmd

    refer = np.ascontiguousarray(np.asarray(refer), dtype=np.float32)
    x = np.ascontiguousarray(np.asarray(x), dtype=np.float32)
    refer8 = refer.astype(ml_dtypes.float8_e4m3)

    if "nc" not in _CACHE:
        _CACHE["nc"] = _build_nc()
    nc = _CACHE["nc"]

    refw, wx, w3m, wf = _host_prep_weights(w_seq, b_seq, w1, b1, w2, b2, w3, b3)
    in_maps = []
    for c in range(N_CORES):
        refer_sl, xwc, edge = _host_prep_core(c, refer8, x)
        in_maps.append(dict(
            refer_sl=refer_sl, xwc=xwc, edge=edge, refw=refw, wx=wx, w3=w3m, wf=wf
        ))

    res = run_bass_kernel_spmd(nc, in_maps, core_ids=list(range(N_CORES)))

    final = np.zeros((32768, D_OUT, 1), np.float32)
    for c in range(N_CORES):
        r = res.results[c]["res"]  # (128, 2048)
        final[2048 * c:2048 * (c + 1), :, 0] = r[0:64, :].T
        final[16384 + 2048 * c:16384 + 2048 * (c + 1), :, 0] = r[64:128, :].T
    return final
